# revision 47
# baseline (speedup 1.0000x reference)
"""Colorizer kernel for Trainium2 (8 NeuronCores, SPMD).

out[b,c,y,x] = sum_p softmax_p(corr[b,p,y,x]) * one_hot(labels)[c, y+dy, x+dx]
over a 13x13 displacement window; corr = <feats_t[:,y,x], feats_r[:,y+dy,x+dx]>
over 256 channels; out-of-bounds displacements get zero weight.

Sharding: core = half*4 + batch. Each core: 64 query rows. The bottom half is
y-MIRRORED on host so all 8 cores run one identical SPMD program (the 13x13
window and band mask are y-symmetric).

Final design (baseline 259us -> ~79us; rel err 7.5e-3 < 2e-2):
  - feats cast to fp16 on host (halves HBM traffic; fp16 matmul is full PE
    rate); E/mask/oht bf16 (fp16 E overflows: corr max ~74.8, bias -64).
  - t in NATURAL layout [128, 2ch x 64 x 128]; gram matmul moving operands
    are 2D-strided slices (verified on HW) -> no x-window duplication, and
    host staging is a cheap transpose. r keys host-packed block-major with
    both channel chunks side by side -> one DMA per block-row.
  - all input DMAs issue from Sync/Scalar queues (HWDGE; GpSimd DMA is
    software-DGE and lands ~9us late). Out DMAs from Sync.
  - aggregation emits 16 classes + a ones-row denominator (17 partitions);
    normalization (v1 spent 52us in DVE reciprocal) divides on host.
  - 4-row strip PSUM tiles [17,512] (matmul output must stay inside one
    2KB PSUM bank); strips for block-row k-1 aggregate at iteration k xb==1
    when all masks are long done -> no PE stalls; the final iteration's
    strips chase its masks with a 3-xb lag to avoid a tail backlog.
  - PE p-state pre-ramp (dummy matmuls) while the first DMAs land.
  Remaining limits: PE streams 98k columns at a power-throttled ~1.5GHz
  (~58us busy), Scalar exp ~46us, ~6.5us fixed preamble + ~8us teardown.
"""
import sys
sys.path.insert(0, "/opt/trn_rl_repo")

import numpy as np
import ml_dtypes

D, R, C = 4, 6, 16
B, CF, H1, W1 = 4, 256, 128, 128
HALF = 64
NBR = 9             # key block-rows per core (72 key rows)
NXB = 8             # x-blocks per row (16 key cols each)
BIAS = -64.0
EPAD = 576          # E tile stride per x-block (max rows*nx = 560)
OC = 17             # 16 classes + ones (denominator) row

_COMPILED = None
_LAST_RES = None


def _windows():
    out = []
    for k in range(NBR):
        ky0 = 8 * k
        a0n = max(0, ky0 - 6)
        b0n = min(HALF, ky0 + 14)
        row = []
        for xb in range(NXB):
            xlo = max(0, 16 * xb - 6)
            xhi = min(W1, 16 * xb + 22)
            nx = xhi - xlo
            rows = b0n - a0n
            assert rows % 2 == 0 and (rows // 2) * nx <= 512
            row.append(dict(ky0=ky0, a0n=a0n, b0n=b0n, rows=rows,
                            xlo=xlo, nx=nx,
                            n_pieces=1 if rows * nx <= 512 else 2))
        out.append(row)
    return out


WIN = _windows()


def _build():
    import concourse.tile as tile
    import concourse.mybir as mybir
    from concourse import bacc
    from contextlib import ExitStack

    f32 = mybir.dt.float32
    f16 = mybir.dt.float16
    bf16 = mybir.dt.bfloat16
    Exp = mybir.ActivationFunctionType.Exp

    nc = bacc.Bacc("TRN2", target_bir_lowering=False, debug=False, num_devices=8)
    t_d = nc.dram_tensor("t", [128, 2 * HALF * W1], f16, kind="ExternalInput").ap()
    r_d = nc.dram_tensor("r", [128, 2 * NBR * 8 * W1], f16,
                         kind="ExternalInput").ap()
    oht_d = nc.dram_tensor("oht", [128, NBR * NXB * OC], bf16,
                           kind="ExternalInput").ap()
    msk_d = nc.dram_tensor("msk", [128, 32 * 28], bf16, kind="ExternalInput").ap()
    out_d = nc.dram_tensor("out", [OC, HALF, W1], f32, kind="ExternalOutput").ap()

    # strip schedule: strip s = query rows [4s, 4s+4), one PSUM bank [17,512].
    # Strips 2k-2 and 2k-1 fire during iteration k.
    contrib = {}
    for s in range(HALF // 4):
        contrib[s] = [k for k in range(NBR)
                      if WIN[k][0]['a0n'] < 4 * s + 4 and WIN[k][0]['b0n'] > 4 * s]
    strip_after = {s: max(ks) for s, ks in contrib.items()}
    assert all(strip_after[s] == min(s // 2 + 1, NBR - 1)
               for s in range(HALF // 4))

    with tile.TileContext(nc) as tc, ExitStack() as ctx:
        const_p = ctx.enter_context(tc.tile_pool(name="const", bufs=1))
        t_p = ctx.enter_context(tc.tile_pool(name="tbuf", bufs=1))
        r_p = ctx.enter_context(tc.tile_pool(name="rbuf", bufs=3))
        e_p = ctx.enter_context(tc.tile_pool(name="ebuf", bufs=4))
        st_p = ctx.enter_context(tc.tile_pool(name="stage", bufs=2))
        gps = ctx.enter_context(tc.tile_pool(name="gram", bufs=3, space="PSUM"))
        aps = ctx.enter_context(tc.tile_pool(name="aggp", bufs=2, space="PSUM"))

        bias_t = const_p.tile([128, 1], f32)
        nc.vector.memset(bias_t[:], BIAS)

        r_tiles = {}

        def load_r(k, eng=None):
            r_tiles[k] = r_p.tile([128, 2 * 8 * W1], f16, tag="r", name=f"r_{k}")
            (eng or nc.sync).dma_start(
                r_tiles[k][:],
                r_d[:, k * 2 * 8 * W1:(k + 1) * 2 * 8 * W1])

        # first loads all on Sync in arrival-priority order; r0 is split by
        # channel chunk so the first gram's ch0 matmul starts on its half
        t_t = t_p.tile([128, 2 * HALF * W1], f16)

        def load_t(g, eng=None):
            for ch in (0, 1):
                o = ch * HALF * W1 + g * 16 * W1
                (eng or nc.sync).dma_start(
                    t_t[:, o:o + 16 * W1], t_d[:, o:o + 16 * W1])

        r_tiles[0] = r_p.tile([128, 2 * 8 * W1], f16, tag="r", name="r_0")
        nc.sync.dma_start(r_tiles[0][:, 0:8 * W1], r_d[:, 0:8 * W1])
        nc.sync.dma_start(t_t[:, 0:16 * W1], t_d[:, 0:16 * W1])
        nc.sync.dma_start(t_t[:, HALF * W1:HALF * W1 + 16 * W1],
                          t_d[:, HALF * W1:HALF * W1 + 16 * W1])
        nc.sync.dma_start(r_tiles[0][:, 8 * W1:16 * W1],
                          r_d[:, 8 * W1:16 * W1])
        t4 = t_t[:].rearrange("p (c r x) -> p c r x", c=2, r=HALF)

        load_r(1)
        load_t(1)
        msk_t = const_p.tile([128, 32 * 28], bf16)
        nc.sync.dma_start(msk_t[:], msk_d[:])
        for g in range(2, 4):
            load_t(g)
        oht_t = const_p.tile([128, NBR * NXB * OC], bf16)
        nc.sync.dma_start(oht_t[:], oht_d[:])
        msk3 = msk_t[:].rearrange("p (m x) -> p m x", m=32)

        # PE p-state pre-ramp: dummy matmuls on a zeroed tile while the input
        # DMAs land (the PE clock needs ~3us of sustained work to reach max).
        warm_t = const_p.tile([128, 512], f16)
        nc.vector.memset(warm_t[:], 0.0)
        wps = aps.tile([OC, 512], f32, tag="aggps", name="warm_ps")
        for _ in range(5):
            nc.tensor.matmul(wps[0:OC, 0:512], warm_t[:, 0:OC], warm_t[:],
                             start=True, stop=True)

        e_tiles = {}
        strip_state = {}  # s -> [pt, n_done, n_total]

        def emit_agg(s, k, xb):
            w = WIN[k][xb]
            ra = max(w['a0n'], 4 * s)
            rb = min(w['b0n'], 4 * s + 4)
            if ra >= rb:
                return
            nx = w['nx']
            pt = strip_state[s][0]
            pt3 = pt[:].rearrange("p (r x) -> p r x", r=4)
            rhs = e_tiles[k][:, xb * EPAD + (ra - w['a0n']) * nx:
                             xb * EPAD + (rb - w['a0n']) * nx]
            lin = k * NXB + xb
            o = pt3[:, ra - 4 * s:rb - 4 * s, w['xlo']:w['xlo'] + nx]
            st = strip_state[s]
            nc.tensor.matmul(o, oht_t[:, lin * OC:(lin + 1) * OC], rhs,
                             start=(st[1] == 0), stop=(st[1] == st[2] - 1))
            st[1] += 1

        def open_strip(s):
            n = sum(1 for k in contrib[s] for xb in range(NXB)
                    if max(WIN[k][0]['a0n'], 4 * s) < min(WIN[k][0]['b0n'],
                                                          4 * s + 4))
            strip_state[s] = [aps.tile([OC, 512], f32, tag="aggps",
                                       name=f"pt_{s}"), 0, n]

        def close_pair(ss):
            for s in ss:
                st = strip_state[s]
                assert st[1] == st[2], (s, st)
                stg = st_p.tile([OC, 512], f32, tag="stg")
                nc.vector.tensor_copy(stg[:], st[0][:])
                nc.sync.dma_start(
                    out_d[:, 4 * s:4 * s + 4, :],
                    stg[:].rearrange("p (r x) -> p r x", r=4))

        def do_gram(k, xb, r_t):
            w = WIN[k][xb]
            rows, nx, xlo, a0 = w['rows'], w['nx'], w['xlo'], w['a0n']
            ntot = rows * nx
            gp = gps.tile([128, 1024], f32, tag="G")
            if w['n_pieces'] == 1:
                offs = [(0, a0, rows)]
            else:
                h = rows // 2
                offs = [(0, a0, h), (512, a0 + h, h)]
            for ch in (0, 1):
                lhsT = r_t[:, ch * 1024 + 128 * xb:ch * 1024 + 128 * xb + 128]
                for (po, pa, pr) in offs:
                    rhs = t4[:, ch, pa:pa + pr, xlo:xlo + nx]
                    o = gp[:, po:po + pr * nx]
                    nc.tensor.matmul(o, lhsT, rhs, start=(ch == 0),
                                     stop=(ch == 1))
            et = e_tiles[k]
            eo = et[:, xb * EPAD:xb * EPAD + ntot]
            if w['n_pieces'] == 1:
                ei = gp[:, 0:ntot]
            else:
                ei = gp[:].rearrange("p (t h) -> p t h", t=2)[:, :, 0:ntot // 2]
                eo = eo.rearrange("p (t h) -> p t h", t=2)
            nc.scalar.activation(eo, ei, Exp, bias=bias_t[:], scale=1.0)
            m_a = a0 - w['ky0'] + 12
            xr = xlo - (16 * xb - 6)
            e3 = et[:, xb * EPAD:xb * EPAD + ntot].rearrange(
                "p (r x) -> p r x", r=rows)
            nc.vector.tensor_mul(
                e3, e3, msk3[:, m_a:m_a + rows, xr:xr + nx])

        for k in range(NBR):
            if k + 2 < NBR:
                load_r(k + 2)
            r_t = r_tiles.pop(k)
            e_tiles[k] = e_p.tile([128, NXB * EPAD], bf16, tag="E",
                                  name=f"E_{k}")
            # strips whose last contributor is k-1: aggregate them now, all
            # masks are done -> no PE stalls
            prev = [s for s in (2 * k - 4, 2 * k - 3) if s >= 0]
            last = [14, 15] if k == NBR - 1 else []
            for xb in range(NXB):
                do_gram(k, xb, r_t)
                if xb == 1:
                    for s in prev:
                        open_strip(s)
                        for kk in contrib[s]:
                            for x2 in range(NXB):
                                emit_agg(s, kk, x2)
                    if prev:
                        close_pair(prev)
                if last:
                    if xb == 2:
                        # final iteration: the very last strips chase this
                        # iteration's masks so they don't pile up at the end
                        for s in last:
                            open_strip(s)
                            for kk in contrib[s]:
                                if kk < k:
                                    for x2 in range(NXB):
                                        emit_agg(s, kk, x2)
                    elif xb >= 3:
                        for s in last:
                            emit_agg(s, k, xb - 3)
            for s in last:
                for x2 in range(NXB - 3, NXB):
                    emit_agg(s, k, x2)
            if last:
                close_pair(last)
    nc.compile()
    return nc


def _prep_host(quantized_r):
    q = quantized_r[:, 0]
    a = q.reshape(B, H1, 4, 512)[:, :, 1:3, :].sum(2)
    s = a.reshape(B, H1, W1, 4)[:, :, :, 1:3].sum(3)
    # CPU-jax reference semantics: f32->i32 convert truncates (values >= 0)
    return s // 4


def _mask_host():
    ky = (np.arange(128) // 16)[:, None, None]
    kx = (np.arange(128) % 16)[:, None, None]
    mi = np.arange(32)[None, :, None]
    rx = np.arange(28)[None, None, :]
    m = ((np.abs(mi - 12 - ky) <= 6) & (np.abs(rx - 6 - kx) <= 6))
    return m.astype(np.float32).reshape(128, 32 * 28).astype(ml_dtypes.bfloat16)


def _oht_host(labels_half):
    o = np.zeros((128, NBR * NXB, OC), np.float32)
    for k in range(NBR):
        for xb in range(NXB):
            lab = labels_half[8 * k:8 * k + 8, 16 * xb:16 * xb + 16].reshape(128)
            o[np.arange(128), k * NXB + xb, lab] = 1.0
            o[:, k * NXB + xb, 16] = 1.0  # denominator ones row
    return o.reshape(128, NBR * NXB * OC).astype(ml_dtypes.bfloat16)


def kernel(feats_r, feats_t, quantized_r):
    global _COMPILED, _LAST_RES
    from concourse.bass_utils import run_bass_kernel_spmd

    feats_r = np.asarray(feats_r, np.float32)
    feats_t = np.asarray(feats_t, np.float32)
    quantized_r = np.asarray(quantized_r, np.int32)

    if _COMPILED is None:
        _COMPILED = _build()

    labels = _prep_host(quantized_r)
    msk = _mask_host()
    fr16 = feats_r.astype(np.float16)
    ft16 = feats_t.astype(np.float16)
    in_maps = []
    for core in range(8):
        half, b = core // 4, core % 4
        if half == 0:
            t = ft16[b, :, 0:HALF, :]
            r = fr16[b, :, 0:72, :]
            lab = labels[b, 0:72, :]
        else:  # y-mirrored bottom half
            t = ft16[b, :, ::-1, :][:, 0:HALF, :]
            r = fr16[b, :, ::-1, :][:, 0:72, :]
            lab = labels[b, ::-1, :][0:72, :]
        # t: [256,64,128] -> [128 part, ch, 64, 128]
        t_n = np.ascontiguousarray(t).reshape(2, 128, HALF, W1) \
            .transpose(1, 0, 2, 3).reshape(128, 2 * HALF * W1)
        # r block-major keys, both channel chunks side by side per block-row:
        # [128 part, k, ch, xb, ky, kx]
        r_bm = np.ascontiguousarray(r).reshape(2, 128, NBR, 8, NXB, 16) \
            .transpose(1, 2, 0, 4, 3, 5).reshape(128, 2 * NBR * 8 * W1)
        in_maps.append(dict(
            t=t_n,
            r=np.ascontiguousarray(r_bm),
            oht=np.ascontiguousarray(_oht_host(lab)),
            msk=msk,
        ))
    res = run_bass_kernel_spmd(_COMPILED, in_maps, core_ids=list(range(8)))
    _LAST_RES = res
    out = np.empty((B, C, H1, W1), np.float32)
    for core in range(8):
        half, b = core // 4, core % 4
        o = res.results[core]["out"]
        o = o[0:16] / o[16:17]
        if half == 0:
            out[b, :, 0:HALF, :] = o
        else:
            out[b, :, HALF:, :] = o[:, ::-1, :]
    return out


# revision 49
# speedup vs baseline: 1.0193x; 1.0193x over previous
"""Colorizer kernel for Trainium2 (8 NeuronCores, SPMD).

out[b,c,y,x] = sum_p softmax_p(corr[b,p,y,x]) * one_hot(labels)[c, y+dy, x+dx]
over a 13x13 displacement window; corr = <feats_t[:,y,x], feats_r[:,y+dy,x+dx]>
over 256 channels; out-of-bounds displacements get zero weight.

Sharding: core = half*4 + batch. Each core: 64 query rows. The bottom half is
y-MIRRORED on host so all 8 cores run one identical SPMD program (the 13x13
window and band mask are y-symmetric).

Final design (baseline 259us -> ~79us; rel err 7.5e-3 < 2e-2):
  - feats cast to fp16 on host (halves HBM traffic; fp16 matmul is full PE
    rate); E/mask/oht bf16 (fp16 E overflows: corr max ~74.8, bias -64).
  - t in NATURAL layout [128, 2ch x 64 x 128]; gram matmul moving operands
    are 2D-strided slices (verified on HW) -> no x-window duplication, and
    host staging is a cheap transpose. r keys host-packed block-major with
    both channel chunks side by side -> one DMA per block-row.
  - all input DMAs issue from Sync/Scalar queues (HWDGE; GpSimd DMA is
    software-DGE and lands ~9us late). Out DMAs from Sync.
  - aggregation emits 16 classes + a ones-row denominator (17 partitions);
    normalization (v1 spent 52us in DVE reciprocal) divides on host.
  - 4-row strip PSUM tiles [17,512] (matmul output must stay inside one
    2KB PSUM bank); strips for block-row k-1 aggregate at iteration k xb==1
    when all masks are long done -> no PE stalls; the final iteration's
    strips chase its masks with a 3-xb lag to avoid a tail backlog.
  - PE p-state pre-ramp (dummy matmuls) while the first DMAs land.
  Remaining limits: PE streams 98k columns at a power-throttled ~1.5GHz
  (~58us busy), Scalar exp ~46us, ~6.5us fixed preamble + ~8us teardown.
"""
import sys
sys.path.insert(0, "/opt/trn_rl_repo")

import numpy as np
import ml_dtypes

D, R, C = 4, 6, 16
B, CF, H1, W1 = 4, 256, 128, 128
HALF = 64
NBR = 9             # key block-rows per core (72 key rows)
NXB = 8             # x-blocks per row (16 key cols each)
BIAS = -64.0
EPAD = 576          # E tile stride per x-block (max rows*nx = 560)
OC = 17             # 16 classes + ones (denominator) row

_COMPILED = None
_LAST_RES = None


def _windows():
    out = []
    for k in range(NBR):
        ky0 = 8 * k
        a0n = max(0, ky0 - 6)
        b0n = min(HALF, ky0 + 14)
        row = []
        for xb in range(NXB):
            xlo = max(0, 16 * xb - 6)
            xhi = min(W1, 16 * xb + 22)
            nx = xhi - xlo
            rows = b0n - a0n
            assert rows % 2 == 0 and (rows // 2) * nx <= 512
            row.append(dict(ky0=ky0, a0n=a0n, b0n=b0n, rows=rows,
                            xlo=xlo, nx=nx,
                            n_pieces=1 if rows * nx <= 512 else 2))
        out.append(row)
    return out


WIN = _windows()


def _build():
    import concourse.tile as tile
    import concourse.mybir as mybir
    from concourse import bacc
    from contextlib import ExitStack

    f32 = mybir.dt.float32
    f16 = mybir.dt.float16
    bf16 = mybir.dt.bfloat16
    Exp = mybir.ActivationFunctionType.Exp

    nc = bacc.Bacc("TRN2", target_bir_lowering=False, debug=False, num_devices=8)
    t_d = nc.dram_tensor("t", [128, 2 * HALF * W1], f16, kind="ExternalInput").ap()
    r_d = nc.dram_tensor("r", [128, 2 * NBR * 8 * W1], f16,
                         kind="ExternalInput").ap()
    oht_d = nc.dram_tensor("oht", [128, NBR * NXB * OC], bf16,
                           kind="ExternalInput").ap()
    msk_d = nc.dram_tensor("msk", [128, 32 * 28], bf16, kind="ExternalInput").ap()
    out_d = nc.dram_tensor("out", [OC, HALF, W1], f32, kind="ExternalOutput").ap()

    # strip schedule: strip s = query rows [4s, 4s+4), one PSUM bank [17,512].
    # Strips 2k-2 and 2k-1 fire during iteration k.
    contrib = {}
    for s in range(HALF // 4):
        contrib[s] = [k for k in range(NBR)
                      if WIN[k][0]['a0n'] < 4 * s + 4 and WIN[k][0]['b0n'] > 4 * s]
    strip_after = {s: max(ks) for s, ks in contrib.items()}
    assert all(strip_after[s] == min(s // 2 + 1, NBR - 1)
               for s in range(HALF // 4))

    with tile.TileContext(nc) as tc, ExitStack() as ctx:
        const_p = ctx.enter_context(tc.tile_pool(name="const", bufs=1))
        t_p = ctx.enter_context(tc.tile_pool(name="tbuf", bufs=1))
        r_p = ctx.enter_context(tc.tile_pool(name="rbuf", bufs=3))
        e_p = ctx.enter_context(tc.tile_pool(name="ebuf", bufs=4))
        st_p = ctx.enter_context(tc.tile_pool(name="stage", bufs=2))
        gps = ctx.enter_context(tc.tile_pool(name="gram", bufs=3, space="PSUM"))
        aps = ctx.enter_context(tc.tile_pool(name="aggp", bufs=2, space="PSUM"))

        bias_t = const_p.tile([128, 1], f32)
        nc.vector.memset(bias_t[:], BIAS)

        r_tiles = {}

        def load_r(k, eng=None):
            r_tiles[k] = r_p.tile([128, 2 * 8 * W1], f16, tag="r", name=f"r_{k}")
            (eng or nc.sync).dma_start(
                r_tiles[k][:],
                r_d[:, k * 2 * 8 * W1:(k + 1) * 2 * 8 * W1])

        # first loads all on Sync in arrival-priority order; r0 is split by
        # channel chunk so the first gram's ch0 matmul starts on its half
        t_t = t_p.tile([128, 2 * HALF * W1], f16)

        def load_t(g, eng=None):
            for ch in (0, 1):
                o = ch * HALF * W1 + g * 16 * W1
                (eng or nc.sync).dma_start(
                    t_t[:, o:o + 16 * W1], t_d[:, o:o + 16 * W1])

        r_tiles[0] = r_p.tile([128, 2 * 8 * W1], f16, tag="r", name="r_0")
        nc.sync.dma_start(r_tiles[0][:, 0:8 * W1], r_d[:, 0:8 * W1])
        nc.sync.dma_start(t_t[:, 0:16 * W1], t_d[:, 0:16 * W1])
        nc.sync.dma_start(t_t[:, HALF * W1:HALF * W1 + 16 * W1],
                          t_d[:, HALF * W1:HALF * W1 + 16 * W1])
        nc.sync.dma_start(r_tiles[0][:, 8 * W1:16 * W1],
                          r_d[:, 8 * W1:16 * W1])
        t4 = t_t[:].rearrange("p (c r x) -> p c r x", c=2, r=HALF)

        # remaining loads in need order: r1/t1 (k=1), msk (first mask ~9us),
        # r2 + oht (first agg, k=2), then the later t slabs
        load_r(1)
        load_t(1)
        msk_t = const_p.tile([128, 32 * 28], bf16)
        nc.sync.dma_start(msk_t[:], msk_d[:])
        load_r(2)
        oht_t = const_p.tile([128, NBR * NXB * OC], bf16)
        nc.sync.dma_start(oht_t[:], oht_d[:])
        for g in range(2, 4):
            load_t(g)
        msk3 = msk_t[:].rearrange("p (m x) -> p m x", m=32)

        # PE p-state pre-ramp: dummy matmuls on a zeroed tile while the input
        # DMAs land (the PE clock needs ~3us of sustained work to reach max).
        warm_t = const_p.tile([128, 512], f16)
        nc.vector.memset(warm_t[:], 0.0)
        wps = aps.tile([OC, 512], f32, tag="aggps", name="warm_ps")
        for _ in range(5):
            nc.tensor.matmul(wps[0:OC, 0:512], warm_t[:, 0:OC], warm_t[:],
                             start=True, stop=True)

        e_tiles = {}
        strip_state = {}  # s -> [pt, n_done, n_total]

        def emit_agg(s, k, xb):
            w = WIN[k][xb]
            ra = max(w['a0n'], 4 * s)
            rb = min(w['b0n'], 4 * s + 4)
            if ra >= rb:
                return
            nx = w['nx']
            pt = strip_state[s][0]
            pt3 = pt[:].rearrange("p (r x) -> p r x", r=4)
            rhs = e_tiles[k][:, xb * EPAD + (ra - w['a0n']) * nx:
                             xb * EPAD + (rb - w['a0n']) * nx]
            lin = k * NXB + xb
            o = pt3[:, ra - 4 * s:rb - 4 * s, w['xlo']:w['xlo'] + nx]
            st = strip_state[s]
            nc.tensor.matmul(o, oht_t[:, lin * OC:(lin + 1) * OC], rhs,
                             start=(st[1] == 0), stop=(st[1] == st[2] - 1))
            st[1] += 1

        def open_strip(s):
            n = sum(1 for k in contrib[s] for xb in range(NXB)
                    if max(WIN[k][0]['a0n'], 4 * s) < min(WIN[k][0]['b0n'],
                                                          4 * s + 4))
            strip_state[s] = [aps.tile([OC, 512], f32, tag="aggps",
                                       name=f"pt_{s}"), 0, n]

        def close_pair(ss):
            for s in ss:
                st = strip_state[s]
                assert st[1] == st[2], (s, st)
                stg = st_p.tile([OC, 512], f32, tag="stg")
                nc.vector.tensor_copy(stg[:], st[0][:])
                nc.sync.dma_start(
                    out_d[:, 4 * s:4 * s + 4, :],
                    stg[:].rearrange("p (r x) -> p r x", r=4))

        def do_gram(k, xb, r_t):
            w = WIN[k][xb]
            rows, nx, xlo, a0 = w['rows'], w['nx'], w['xlo'], w['a0n']
            ntot = rows * nx
            gp = gps.tile([128, 1024], f32, tag="G")
            if w['n_pieces'] == 1:
                offs = [(0, a0, rows)]
            else:
                h = rows // 2
                offs = [(0, a0, h), (512, a0 + h, h)]
            for ch in (0, 1):
                lhsT = r_t[:, ch * 1024 + 128 * xb:ch * 1024 + 128 * xb + 128]
                for (po, pa, pr) in offs:
                    rhs = t4[:, ch, pa:pa + pr, xlo:xlo + nx]
                    o = gp[:, po:po + pr * nx]
                    nc.tensor.matmul(o, lhsT, rhs, start=(ch == 0),
                                     stop=(ch == 1))
            et = e_tiles[k]
            eo = et[:, xb * EPAD:xb * EPAD + ntot]
            if w['n_pieces'] == 1:
                ei = gp[:, 0:ntot]
            else:
                ei = gp[:].rearrange("p (t h) -> p t h", t=2)[:, :, 0:ntot // 2]
                eo = eo.rearrange("p (t h) -> p t h", t=2)
            nc.scalar.activation(eo, ei, Exp, bias=bias_t[:], scale=1.0)
            m_a = a0 - w['ky0'] + 12
            xr = xlo - (16 * xb - 6)
            e3 = et[:, xb * EPAD:xb * EPAD + ntot].rearrange(
                "p (r x) -> p r x", r=rows)
            nc.vector.tensor_mul(
                e3, e3, msk3[:, m_a:m_a + rows, xr:xr + nx])

        for k in range(NBR):
            if k >= 1 and k + 2 < NBR:
                load_r(k + 2)
            r_t = r_tiles.pop(k)
            e_tiles[k] = e_p.tile([128, NXB * EPAD], bf16, tag="E",
                                  name=f"E_{k}")
            # strips whose last contributor is k-1: aggregate them now, all
            # masks are done -> no PE stalls
            prev = [s for s in (2 * k - 4, 2 * k - 3) if s >= 0]
            last = [14, 15] if k == NBR - 1 else []
            for xb in range(NXB):
                do_gram(k, xb, r_t)
                if xb == 1:
                    for s in prev:
                        open_strip(s)
                        for kk in contrib[s]:
                            for x2 in range(NXB):
                                emit_agg(s, kk, x2)
                    if prev:
                        close_pair(prev)
                if last:
                    if xb == 2:
                        # final iteration: the very last strips chase this
                        # iteration's masks so they don't pile up at the end
                        for s in last:
                            open_strip(s)
                            for kk in contrib[s]:
                                if kk < k:
                                    for x2 in range(NXB):
                                        emit_agg(s, kk, x2)
                    elif xb >= 3:
                        for s in last:
                            emit_agg(s, k, xb - 3)
            for s in last:
                for x2 in range(NXB - 3, NXB):
                    emit_agg(s, k, x2)
            if last:
                close_pair(last)
    nc.compile()
    return nc


def _prep_host(quantized_r):
    q = quantized_r[:, 0]
    a = q.reshape(B, H1, 4, 512)[:, :, 1:3, :].sum(2)
    s = a.reshape(B, H1, W1, 4)[:, :, :, 1:3].sum(3)
    # CPU-jax reference semantics: f32->i32 convert truncates (values >= 0)
    return s // 4


def _mask_host():
    ky = (np.arange(128) // 16)[:, None, None]
    kx = (np.arange(128) % 16)[:, None, None]
    mi = np.arange(32)[None, :, None]
    rx = np.arange(28)[None, None, :]
    m = ((np.abs(mi - 12 - ky) <= 6) & (np.abs(rx - 6 - kx) <= 6))
    return m.astype(np.float32).reshape(128, 32 * 28).astype(ml_dtypes.bfloat16)


def _oht_host(labels_half):
    o = np.zeros((128, NBR * NXB, OC), np.float32)
    for k in range(NBR):
        for xb in range(NXB):
            lab = labels_half[8 * k:8 * k + 8, 16 * xb:16 * xb + 16].reshape(128)
            o[np.arange(128), k * NXB + xb, lab] = 1.0
            o[:, k * NXB + xb, 16] = 1.0  # denominator ones row
    return o.reshape(128, NBR * NXB * OC).astype(ml_dtypes.bfloat16)


def kernel(feats_r, feats_t, quantized_r):
    global _COMPILED, _LAST_RES
    from concourse.bass_utils import run_bass_kernel_spmd

    feats_r = np.asarray(feats_r, np.float32)
    feats_t = np.asarray(feats_t, np.float32)
    quantized_r = np.asarray(quantized_r, np.int32)

    if _COMPILED is None:
        _COMPILED = _build()

    labels = _prep_host(quantized_r)
    msk = _mask_host()
    fr16 = feats_r.astype(np.float16)
    ft16 = feats_t.astype(np.float16)
    in_maps = []
    for core in range(8):
        half, b = core // 4, core % 4
        if half == 0:
            t = ft16[b, :, 0:HALF, :]
            r = fr16[b, :, 0:72, :]
            lab = labels[b, 0:72, :]
        else:  # y-mirrored bottom half
            t = ft16[b, :, ::-1, :][:, 0:HALF, :]
            r = fr16[b, :, ::-1, :][:, 0:72, :]
            lab = labels[b, ::-1, :][0:72, :]
        # t: [256,64,128] -> [128 part, ch, 64, 128]
        t_n = np.ascontiguousarray(t).reshape(2, 128, HALF, W1) \
            .transpose(1, 0, 2, 3).reshape(128, 2 * HALF * W1)
        # r block-major keys, both channel chunks side by side per block-row:
        # [128 part, k, ch, xb, ky, kx]
        r_bm = np.ascontiguousarray(r).reshape(2, 128, NBR, 8, NXB, 16) \
            .transpose(1, 2, 0, 4, 3, 5).reshape(128, 2 * NBR * 8 * W1)
        in_maps.append(dict(
            t=t_n,
            r=np.ascontiguousarray(r_bm),
            oht=np.ascontiguousarray(_oht_host(lab)),
            msk=msk,
        ))
    res = run_bass_kernel_spmd(_COMPILED, in_maps, core_ids=list(range(8)))
    _LAST_RES = res
    out = np.empty((B, C, H1, W1), np.float32)
    for core in range(8):
        half, b = core // 4, core % 4
        o = res.results[core]["out"]
        o = o[0:16] / o[16:17]
        if half == 0:
            out[b, :, 0:HALF, :] = o
        else:
            out[b, :, HALF:, :] = o[:, ::-1, :]
    return out


# revision 50
# speedup vs baseline: 1.0247x; 1.0053x over previous
"""Colorizer kernel for Trainium2 (8 NeuronCores, SPMD).

out[b,c,y,x] = sum_p softmax_p(corr[b,p,y,x]) * one_hot(labels)[c, y+dy, x+dx]
over a 13x13 displacement window; corr = <feats_t[:,y,x], feats_r[:,y+dy,x+dx]>
over 256 channels; out-of-bounds displacements get zero weight.

Sharding: core = half*4 + batch. Each core: 64 query rows. The bottom half is
y-MIRRORED on host so all 8 cores run one identical SPMD program (the 13x13
window and band mask are y-symmetric).

Final design (baseline 259us -> ~79us; rel err 7.5e-3 < 2e-2):
  - feats cast to fp16 on host (halves HBM traffic; fp16 matmul is full PE
    rate); E/mask/oht bf16 (fp16 E overflows: corr max ~74.8, bias -64).
  - t in NATURAL layout [128, 2ch x 64 x 128]; gram matmul moving operands
    are 2D-strided slices (verified on HW) -> no x-window duplication, and
    host staging is a cheap transpose. r keys host-packed block-major with
    both channel chunks side by side -> one DMA per block-row.
  - all input DMAs issue from Sync/Scalar queues (HWDGE; GpSimd DMA is
    software-DGE and lands ~9us late). Out DMAs from Sync.
  - aggregation emits 16 classes + a ones-row denominator (17 partitions);
    normalization (v1 spent 52us in DVE reciprocal) divides on host.
  - 4-row strip PSUM tiles [17,512] (matmul output must stay inside one
    2KB PSUM bank); strips for block-row k-1 aggregate at iteration k xb==1
    when all masks are long done -> no PE stalls; the final iteration's
    strips chase its masks with a 3-xb lag to avoid a tail backlog.
  - PE p-state pre-ramp (dummy matmuls) while the first DMAs land.
  Remaining limits: PE streams 98k columns at a power-throttled ~1.5GHz
  (~58us busy), Scalar exp ~46us, ~6.5us fixed preamble + ~8us teardown.
"""
import sys
sys.path.insert(0, "/opt/trn_rl_repo")

import numpy as np
import ml_dtypes

D, R, C = 4, 6, 16
B, CF, H1, W1 = 4, 256, 128, 128
HALF = 64
NBR = 9             # key block-rows per core (72 key rows)
NXB = 8             # x-blocks per row (16 key cols each)
BIAS = -64.0
EPAD = 576          # E tile stride per x-block (max rows*nx = 560)
OC = 17             # 16 classes + ones (denominator) row

_COMPILED = None
_LAST_RES = None


def _windows():
    out = []
    for k in range(NBR):
        ky0 = 8 * k
        a0n = max(0, ky0 - 6)
        b0n = min(HALF, ky0 + 14)
        row = []
        for xb in range(NXB):
            xlo = max(0, 16 * xb - 6)
            xhi = min(W1, 16 * xb + 22)
            nx = xhi - xlo
            rows = b0n - a0n
            assert rows % 2 == 0 and (rows // 2) * nx <= 512
            row.append(dict(ky0=ky0, a0n=a0n, b0n=b0n, rows=rows,
                            xlo=xlo, nx=nx,
                            n_pieces=1 if rows * nx <= 512 else 2))
        out.append(row)
    return out


WIN = _windows()


def _build():
    import concourse.tile as tile
    import concourse.mybir as mybir
    from concourse import bacc
    from contextlib import ExitStack

    f32 = mybir.dt.float32
    f16 = mybir.dt.float16
    bf16 = mybir.dt.bfloat16
    Exp = mybir.ActivationFunctionType.Exp

    nc = bacc.Bacc("TRN2", target_bir_lowering=False, debug=False, num_devices=8)
    t_d = nc.dram_tensor("t", [128, 2 * HALF * W1], f16, kind="ExternalInput").ap()
    r_d = nc.dram_tensor("r", [128, 2 * NBR * 8 * W1], f16,
                         kind="ExternalInput").ap()
    oht_d = nc.dram_tensor("oht", [128, NBR * NXB * OC], bf16,
                           kind="ExternalInput").ap()
    msk_d = nc.dram_tensor("msk", [128, 32 * 28], bf16, kind="ExternalInput").ap()
    out_d = nc.dram_tensor("out", [OC, HALF, W1], f32, kind="ExternalOutput").ap()

    # strip schedule: strip s = query rows [4s, 4s+4), one PSUM bank [17,512].
    # Strips 2k-2 and 2k-1 fire during iteration k.
    contrib = {}
    for s in range(HALF // 4):
        contrib[s] = [k for k in range(NBR)
                      if WIN[k][0]['a0n'] < 4 * s + 4 and WIN[k][0]['b0n'] > 4 * s]
    strip_after = {s: max(ks) for s, ks in contrib.items()}
    assert all(strip_after[s] == min(s // 2 + 1, NBR - 1)
               for s in range(HALF // 4))

    with tile.TileContext(nc) as tc, ExitStack() as ctx:
        const_p = ctx.enter_context(tc.tile_pool(name="const", bufs=1))
        t_p = ctx.enter_context(tc.tile_pool(name="tbuf", bufs=1))
        r_p = ctx.enter_context(tc.tile_pool(name="rbuf", bufs=3))
        e_p = ctx.enter_context(tc.tile_pool(name="ebuf", bufs=4))
        st_p = ctx.enter_context(tc.tile_pool(name="stage", bufs=2))
        gps = ctx.enter_context(tc.tile_pool(name="gram", bufs=3, space="PSUM"))
        aps = ctx.enter_context(tc.tile_pool(name="aggp", bufs=2, space="PSUM"))

        bias_t = const_p.tile([128, 1], f32)
        nc.vector.memset(bias_t[:], BIAS)

        r_tiles = {}

        def load_r(k, eng=None):
            r_tiles[k] = r_p.tile([128, 2 * 8 * W1], f16, tag="r", name=f"r_{k}")
            (eng or nc.sync).dma_start(
                r_tiles[k][:],
                r_d[:, k * 2 * 8 * W1:(k + 1) * 2 * 8 * W1])

        # first loads all on Sync in arrival-priority order; r0 is split by
        # channel chunk so the first gram's ch0 matmul starts on its half
        t_t = t_p.tile([128, 2 * HALF * W1], f16)

        def load_t(g, eng=None):
            for ch in (0, 1):
                o = ch * HALF * W1 + g * 16 * W1
                (eng or nc.sync).dma_start(
                    t_t[:, o:o + 16 * W1], t_d[:, o:o + 16 * W1])

        r_tiles[0] = r_p.tile([128, 2 * 8 * W1], f16, tag="r", name="r_0")
        nc.sync.dma_start(r_tiles[0][:, 0:8 * W1], r_d[:, 0:8 * W1])
        nc.sync.dma_start(t_t[:, 0:16 * W1], t_d[:, 0:16 * W1])
        nc.sync.dma_start(t_t[:, HALF * W1:HALF * W1 + 16 * W1],
                          t_d[:, HALF * W1:HALF * W1 + 16 * W1])
        nc.sync.dma_start(r_tiles[0][:, 8 * W1:16 * W1],
                          r_d[:, 8 * W1:16 * W1])
        t4 = t_t[:].rearrange("p (c r x) -> p c r x", c=2, r=HALF)

        # remaining loads in need order: r1/t1 (k=1), msk (first mask ~9us),
        # r2 + oht (first agg, k=2), then the later t slabs
        load_r(1)
        load_t(1)
        msk_t = const_p.tile([128, 32 * 28], bf16)
        nc.sync.dma_start(msk_t[:], msk_d[:])
        load_r(2)
        oht_t = const_p.tile([128, NBR * NXB * OC], bf16)
        nc.sync.dma_start(oht_t[:], oht_d[:])
        for g in range(2, 4):
            load_t(g)
        msk3 = msk_t[:].rearrange("p (m x) -> p m x", m=32)

        # PE p-state pre-ramp: dummy matmuls on a zeroed tile while the input
        # DMAs land (the PE clock needs ~3us of sustained work to reach max).
        warm_t = const_p.tile([128, 512], f16)
        nc.vector.memset(warm_t[:], 0.0)
        wps = aps.tile([OC, 512], f32, tag="aggps", name="warm_ps")
        for _ in range(3):
            nc.tensor.matmul(wps[0:OC, 0:512], warm_t[:, 0:OC], warm_t[:],
                             start=True, stop=True)

        e_tiles = {}
        strip_state = {}  # s -> [pt, n_done, n_total]

        def emit_agg(s, k, xb):
            w = WIN[k][xb]
            ra = max(w['a0n'], 4 * s)
            rb = min(w['b0n'], 4 * s + 4)
            if ra >= rb:
                return
            nx = w['nx']
            pt = strip_state[s][0]
            pt3 = pt[:].rearrange("p (r x) -> p r x", r=4)
            rhs = e_tiles[k][:, xb * EPAD + (ra - w['a0n']) * nx:
                             xb * EPAD + (rb - w['a0n']) * nx]
            lin = k * NXB + xb
            o = pt3[:, ra - 4 * s:rb - 4 * s, w['xlo']:w['xlo'] + nx]
            st = strip_state[s]
            nc.tensor.matmul(o, oht_t[:, lin * OC:(lin + 1) * OC], rhs,
                             start=(st[1] == 0), stop=(st[1] == st[2] - 1))
            st[1] += 1

        def open_strip(s):
            n = sum(1 for k in contrib[s] for xb in range(NXB)
                    if max(WIN[k][0]['a0n'], 4 * s) < min(WIN[k][0]['b0n'],
                                                          4 * s + 4))
            strip_state[s] = [aps.tile([OC, 512], f32, tag="aggps",
                                       name=f"pt_{s}"), 0, n]

        def close_pair(ss):
            for s in ss:
                st = strip_state[s]
                assert st[1] == st[2], (s, st)
                stg = st_p.tile([OC, 512], f32, tag="stg")
                nc.vector.tensor_copy(stg[:], st[0][:])
                nc.sync.dma_start(
                    out_d[:, 4 * s:4 * s + 4, :],
                    stg[:].rearrange("p (r x) -> p r x", r=4))

        def do_gram(k, xb, r_t):
            w = WIN[k][xb]
            rows, nx, xlo, a0 = w['rows'], w['nx'], w['xlo'], w['a0n']
            ntot = rows * nx
            gp = gps.tile([128, 1024], f32, tag="G")
            if w['n_pieces'] == 1:
                offs = [(0, a0, rows)]
            else:
                h = rows // 2
                offs = [(0, a0, h), (512, a0 + h, h)]
            for ch in (0, 1):
                lhsT = r_t[:, ch * 1024 + 128 * xb:ch * 1024 + 128 * xb + 128]
                for (po, pa, pr) in offs:
                    rhs = t4[:, ch, pa:pa + pr, xlo:xlo + nx]
                    o = gp[:, po:po + pr * nx]
                    nc.tensor.matmul(o, lhsT, rhs, start=(ch == 0),
                                     stop=(ch == 1))
            et = e_tiles[k]
            eo = et[:, xb * EPAD:xb * EPAD + ntot]
            if w['n_pieces'] == 1:
                ei = gp[:, 0:ntot]
            else:
                ei = gp[:].rearrange("p (t h) -> p t h", t=2)[:, :, 0:ntot // 2]
                eo = eo.rearrange("p (t h) -> p t h", t=2)
            nc.scalar.activation(eo, ei, Exp, bias=bias_t[:], scale=1.0)
            m_a = a0 - w['ky0'] + 12
            xr = xlo - (16 * xb - 6)
            e3 = et[:, xb * EPAD:xb * EPAD + ntot].rearrange(
                "p (r x) -> p r x", r=rows)
            nc.vector.tensor_mul(
                e3, e3, msk3[:, m_a:m_a + rows, xr:xr + nx])

        for k in range(NBR):
            if k >= 1 and k + 2 < NBR:
                load_r(k + 2)
            r_t = r_tiles.pop(k)
            e_tiles[k] = e_p.tile([128, NXB * EPAD], bf16, tag="E",
                                  name=f"E_{k}")
            # strips whose last contributor is k-1: aggregate them now, all
            # masks are done -> no PE stalls
            prev = [s for s in (2 * k - 4, 2 * k - 3) if s >= 0]
            last = [14, 15] if k == NBR - 1 else []
            for xb in range(NXB):
                do_gram(k, xb, r_t)
                if xb == 1:
                    for s in prev:
                        open_strip(s)
                        for kk in contrib[s]:
                            for x2 in range(NXB):
                                emit_agg(s, kk, x2)
                    if prev:
                        close_pair(prev)
                if last:
                    if xb == 2:
                        # final iteration: the very last strips chase this
                        # iteration's masks so they don't pile up at the end
                        for s in last:
                            open_strip(s)
                            for kk in contrib[s]:
                                if kk < k:
                                    for x2 in range(NXB):
                                        emit_agg(s, kk, x2)
                    elif xb >= 3:
                        for s in last:
                            emit_agg(s, k, xb - 3)
            for s in last:
                for x2 in range(NXB - 3, NXB):
                    emit_agg(s, k, x2)
            if last:
                close_pair(last)
    nc.compile()
    return nc


def _prep_host(quantized_r):
    q = quantized_r[:, 0]
    a = q.reshape(B, H1, 4, 512)[:, :, 1:3, :].sum(2)
    s = a.reshape(B, H1, W1, 4)[:, :, :, 1:3].sum(3)
    # CPU-jax reference semantics: f32->i32 convert truncates (values >= 0)
    return s // 4


def _mask_host():
    ky = (np.arange(128) // 16)[:, None, None]
    kx = (np.arange(128) % 16)[:, None, None]
    mi = np.arange(32)[None, :, None]
    rx = np.arange(28)[None, None, :]
    m = ((np.abs(mi - 12 - ky) <= 6) & (np.abs(rx - 6 - kx) <= 6))
    return m.astype(np.float32).reshape(128, 32 * 28).astype(ml_dtypes.bfloat16)


def _oht_host(labels_half):
    o = np.zeros((128, NBR * NXB, OC), np.float32)
    for k in range(NBR):
        for xb in range(NXB):
            lab = labels_half[8 * k:8 * k + 8, 16 * xb:16 * xb + 16].reshape(128)
            o[np.arange(128), k * NXB + xb, lab] = 1.0
            o[:, k * NXB + xb, 16] = 1.0  # denominator ones row
    return o.reshape(128, NBR * NXB * OC).astype(ml_dtypes.bfloat16)


def kernel(feats_r, feats_t, quantized_r):
    global _COMPILED, _LAST_RES
    from concourse.bass_utils import run_bass_kernel_spmd

    feats_r = np.asarray(feats_r, np.float32)
    feats_t = np.asarray(feats_t, np.float32)
    quantized_r = np.asarray(quantized_r, np.int32)

    if _COMPILED is None:
        _COMPILED = _build()

    labels = _prep_host(quantized_r)
    msk = _mask_host()
    fr16 = feats_r.astype(np.float16)
    ft16 = feats_t.astype(np.float16)
    in_maps = []
    for core in range(8):
        half, b = core // 4, core % 4
        if half == 0:
            t = ft16[b, :, 0:HALF, :]
            r = fr16[b, :, 0:72, :]
            lab = labels[b, 0:72, :]
        else:  # y-mirrored bottom half
            t = ft16[b, :, ::-1, :][:, 0:HALF, :]
            r = fr16[b, :, ::-1, :][:, 0:72, :]
            lab = labels[b, ::-1, :][0:72, :]
        # t: [256,64,128] -> [128 part, ch, 64, 128]
        t_n = np.ascontiguousarray(t).reshape(2, 128, HALF, W1) \
            .transpose(1, 0, 2, 3).reshape(128, 2 * HALF * W1)
        # r block-major keys, both channel chunks side by side per block-row:
        # [128 part, k, ch, xb, ky, kx]
        r_bm = np.ascontiguousarray(r).reshape(2, 128, NBR, 8, NXB, 16) \
            .transpose(1, 2, 0, 4, 3, 5).reshape(128, 2 * NBR * 8 * W1)
        in_maps.append(dict(
            t=t_n,
            r=np.ascontiguousarray(r_bm),
            oht=np.ascontiguousarray(_oht_host(lab)),
            msk=msk,
        ))
    res = run_bass_kernel_spmd(_COMPILED, in_maps, core_ids=list(range(8)))
    _LAST_RES = res
    out = np.empty((B, C, H1, W1), np.float32)
    for core in range(8):
        half, b = core // 4, core % 4
        o = res.results[core]["out"]
        o = o[0:16] / o[16:17]
        if half == 0:
            out[b, :, 0:HALF, :] = o
        else:
            out[b, :, HALF:, :] = o[:, ::-1, :]
    return out
